# revision 12
# baseline (speedup 1.0000x reference)
"""Trainium2 Bass kernel for nn_DeepAggregateAutoEncoder.

Network: 4 layers, each computing out[b, o] = min_or_max_k x[b, conn[o, k]]
(K=32 random connections per output feature, per-output min/max select).

Strategy (per core, data-parallel over batch, 8 cores):
  - Keep activations feature-major in DRAM: hT[row, batch] with batch=512
    per core, so one "feature row" is a contiguous 2KB line.
  - Each layer is computed as 32 row-gathers (one per connection slot k)
    using the GPSIMD dma_gather instruction (gathered row j lands at
    partition j%128, block j//128), folded together with vector-engine
    min/max tensor_tensor ops.
  - Outputs of each layer are pre-sorted (host-side) into a min-group and a
    max-group, each padded to a multiple of 128 rows, so every fold chain
    uses a single op. The permutation is folded into the next layer's
    (runtime) index table; the final inverse permutation is applied on the
    host during unsharding.
  - Even/odd connection slots fold into two separate accumulator tiles
    (shortens the serial fold chain); one final vector op combines them,
    then the result is written back to DRAM for the next layer's gather.

All indices are runtime tensor data, so the compiled program depends only on
the op-count split per layer (cached across calls).
"""

import numpy as np

K = 32
P = 128
B = 4096
N_IN = 1024
NCORES = 8
BC = B // NCORES  # batch per core

_prog_cache = {}


# ---------------------------------------------------------------- host prep

def _preprocess(conns, ops):
    """Build per-layer gather index tables.

    Layer rows (sorted order): min-outputs first (padded to 128*OBa rows),
    then max-outputs (padded to 128*OBb). Row r of the layer output holds
    sorted output #r; gathered row j lands at dest cell (p=j%128, ob=j//128)
    which is written back to hT row j, so idx arrays are in plain row order.

    Returns (specs, pos_final): specs[l] has OBa/OBb/idx_min/idx_max
    ([K, 128*OB] int16 row numbers into the previous layer's hT) and
    pos_final maps original output feature -> row in the final output.
    """
    pos = np.arange(N_IN, dtype=np.int64)
    specs = []
    for l in range(len(conns)):
        conn = np.asarray(conns[l])
        opv = np.asarray(ops[l])
        out_f = conn.shape[0]
        order = np.argsort(opv, kind="stable")
        n_min = int((opv == 0).sum())
        order_min, order_max = order[:n_min], order[n_min:]
        n_max = out_f - n_min
        OBa = -(-n_min // P) if n_min else 0
        OBb = -(-n_max // P) if n_max else 0
        src = pos[conn]  # [out_f, K] rows in prev hT

        def part_idx(order_part, OB):
            rows = OB * P
            padded = np.concatenate(
                [order_part,
                 np.full(rows - len(order_part), order_part[0],
                         dtype=order_part.dtype)]
            )
            return np.ascontiguousarray(src[padded].T.astype(np.int16))  # [K, rows]

        idx_min = part_idx(order_min, OBa) if OBa else np.zeros((K, 0), np.int16)
        idx_max = part_idx(order_max, OBb) if OBb else np.zeros((K, 0), np.int16)

        pos_new = np.zeros(out_f, dtype=np.int64)
        pos_new[order_min] = np.arange(n_min)
        pos_new[order_max] = OBa * P + np.arange(n_max)
        pos = pos_new
        specs.append(dict(OBa=OBa, OBb=OBb, idx_min=idx_min, idx_max=idx_max,
                          rows=(OBa + OBb) * P))
    return specs, pos


def _pack_idx(specs):
    """Pack idx tables into one [128, TOT16] int16 in dma_gather's wrapped-16
    layout: index j of a gather lives at (partition j%16, col base+j//16),
    replicated across the eight 16-partition groups."""
    cols = []
    for s in specs:
        for nm in ("idx_min", "idx_max"):
            a = s[nm]  # [K, rows]
            if a.shape[1] == 0:
                continue
            for k in range(K):
                blk = a[k].reshape(-1, 16).T          # [16, rows/16]
                cols.append(np.tile(blk, (8, 1)))     # [128, rows/16]
    return np.ascontiguousarray(np.concatenate(cols, axis=1).astype(np.int16))


# ------------------------------------------------------------- bass program

def _build_program(shape_key, nlayers=None, nk=K):
    """shape_key: tuple of (OBa, OBb) per layer. Returns compiled Bacc.
    nlayers/nk truncate the program for debugging."""
    import concourse.bacc as bacc
    import concourse.tile as tile
    import concourse.mybir as mybir
    from contextlib import ExitStack

    f32 = mybir.dt.float32
    i16 = mybir.dt.int16
    TOT16 = sum(K * (oa + ob) * 8 for oa, ob in shape_key)
    if nlayers is None:
        nlayers = len(shape_key)
    rows_last = (shape_key[nlayers - 1][0] + shape_key[nlayers - 1][1]) * P

    nc = bacc.Bacc("TRN2", target_bir_lowering=False, debug=False,
                   num_devices=NCORES)
    xT = nc.dram_tensor("xT", [N_IN, BC], f32, kind="ExternalInput")
    idx_d = nc.dram_tensor("idx", [P, TOT16], i16, kind="ExternalInput")
    outT = nc.dram_tensor("outT", [rows_last, BC], f32, kind="ExternalOutput")
    h_int = [
        nc.dram_tensor(f"h{l}", [(oa + ob) * P, BC], f32)
        for l, (oa, ob) in enumerate(shape_key[:nlayers - 1])
    ]

    with tile.TileContext(nc) as tc, ExitStack() as ctx:
        sb = ctx.enter_context(tc.tile_pool(name="sb", bufs=1))
        idx_t = sb.tile([P, TOT16], i16, tag="idx", name="idxt")
        nc.sync.dma_start(idx_t[:], idx_d.ap())

        prev = xT.ap()
        col = 0
        for l, (OBa, OBb) in enumerate(shape_key[:nlayers]):
            h_out = outT.ap() if l == nlayers - 1 else h_int[l].ap()
            row_off = 0
            for part, OB, alu in (("a", OBa, mybir.AluOpType.min),
                                  ("b", OBb, mybir.AluOpType.max)):
                if OB == 0:
                    continue
                acc = [sb.tile([P, OB * BC], f32, tag=f"acc{j}",
                               name=f"acc{l}{part}{j}")
                       for j in range(2)]

                def gather(dst, k):
                    # dma_gather crashes above 1024 indices per instruction;
                    # split into chunks of <= 8 blocks (verified empirically).
                    for ob0 in range(0, OB, 8):
                        obc = min(8, OB - ob0)
                        c0 = col + k * 8 * OB + ob0 * 8
                        nc.gpsimd.dma_gather(
                            out_ap=dst[:, ob0 * BC: (ob0 + obc) * BC]
                                .rearrange("p (ob b) -> p ob b", b=BC),
                            in_ap=prev,
                            idxs_ap=idx_t[:, c0: c0 + obc * 8],
                            num_idxs=P * obc,
                            num_idxs_reg=P * obc,
                            elem_size=BC,
                        )

                # k=0,1 land directly in the accumulators; k>=2 go through
                # staging tiles and fold in on the vector engine.
                gather(acc[0], 0)
                gather(acc[1], 1)
                for k in range(2, nk):
                    stg = sb.tile([P, OB * BC], f32, tag="stg", bufs=4,
                                  name=f"stg{l}{part}{k}")
                    gather(stg, k)
                    a = acc[k % 2]
                    nc.vector.tensor_tensor(out=a[:], in0=a[:], in1=stg[:],
                                            op=alu)
                nc.vector.tensor_tensor(out=acc[0][:], in0=acc[0][:],
                                        in1=acc[1][:], op=alu)
                # acc cell (p, ob) holds row ob*128+p of this part
                dst_ap = h_out[row_off: row_off + P * OB, :].rearrange(
                    "(ob p) b -> p ob b", p=P)
                nc.sync.dma_start(dst_ap, acc[0][:])
                row_off += P * OB
                col += K * 8 * OB
            prev = h_out
    nc.compile()
    return nc


def _get_program(shape_key):
    if shape_key not in _prog_cache:
        _prog_cache[shape_key] = _build_program(shape_key)
    return _prog_cache[shape_key]


# ------------------------------------------------------------------ driver

def kernel(x, conn0, conn1, conn2, conn3, op0, op1, op2, op3):
    from concourse.bass_utils import run_bass_kernel_spmd

    x = np.asarray(x, dtype=np.float32)
    conns = [np.asarray(c, dtype=np.int64) for c in (conn0, conn1, conn2, conn3)]
    ops = [np.asarray(o) for o in (op0, op1, op2, op3)]

    specs, pos_final = _preprocess(conns, ops)
    idx_packed = _pack_idx(specs)
    shape_key = tuple((s["OBa"], s["OBb"]) for s in specs)
    nc = _get_program(shape_key)

    in_maps = []
    for c in range(NCORES):
        xT = np.ascontiguousarray(x[c * BC:(c + 1) * BC].T)
        in_maps.append({"xT": xT, "idx": idx_packed})

    res = run_bass_kernel_spmd(nc, in_maps, list(range(NCORES)))
    outs = []
    for c in range(NCORES):
        outT = res.results[c]["outT"]          # [rows_last, BC]
        outs.append(np.ascontiguousarray(outT[pos_final].T))  # [BC, 1024]
    return np.concatenate(outs, axis=0)
